# revision 42
# baseline (speedup 1.0000x reference)
"""GPT forward pass on 8 Trainium2 NeuronCores — fp8 attention core + bf16
value/FFN/lm paths, head-parallel attention, two-stream software pipeline.

Sharding:
- Trunk (LN/QKV/Wo/FFN/lm_head) is token-parallel: core r owns q-tile r
  (rows 128r..128r+127) of each of the 4 sequences (512 tokens/core).
- Attention is head-parallel: core r owns heads {2r, 2r+1} for all 4 seqs,
  exchanged via AllToAll (Q/K/V out, O back). This makes causal skipping
  SPMD-uniform: unit attention computes only k-tiles kt <= qt and the
  diagonal mask sits at a static slot (one constant triangular multiply).
- The batch is split into two independent streams (seqs {0,1} / {2,3})
  interleaved through each layer so one stream's compute hides the other
  stream's AllToAll latency.
- lm_head is token-sharded: each core computes its 512 tokens against the
  full (padded) vocab, weights streamed, no collective. Logits exit fp16;
  blm is added on host.

Precision (validated against a numpy emulation of the exact quantization
chain; maxrel ~9e-3 vs the 2e-2 gate): Q/K projections and S = K^T Q run in
fp8e4m3 with DoubleRow perf mode (softmax damps their quantization noise).
The value path (V, P@V, Wo), FFN and lm_head run in bf16 — their errors hit
the residual stream and logits directly and fp8 there blows the error
budget. PSUM accumulation is fp32 everywhere; LN stats, softmax and the
residual stream stay fp32. The residual is carried at 256x (LN is
affine-invariant) so Wo/W2 PSUM deltas add in without a rescale pass.
Softmax denominators come free from a ones-column in V; biases enter as
rank-1 (K=1) matmul accumulations; b1 rides the Gelu activation bias.
"""

import os
import sys

for _p in ("/opt/trn_rl_repo",):
    if os.path.isdir(_p) and _p not in sys.path:
        sys.path.insert(0, _p)

import numpy as np
import ml_dtypes

FP8NP = ml_dtypes.float8_e4m3
BF16NP = ml_dtypes.bfloat16

import concourse.bass as bass
import concourse.mybir as mybir
import concourse.tile as tile
from concourse import bacc
from concourse.bass_utils import run_bass_kernel_spmd
from concourse.masks import make_identity

F32 = mybir.dt.float32
BF = mybir.dt.bfloat16
FP8 = mybir.dt.float8e4
FP16 = mybir.dt.float16
AF = mybir.ActivationFunctionType
DR = mybir.MatmulPerfMode.DoubleRow

V, C, T, H, L, B = 32000, 1024, 1024, 16, 4, 4
HD = C // H          # 64
FF = 4 * C           # 4096
NCORES = 8
TL = 512             # local tokens per core (4 seqs x 128)
SEQ = B              # 4
NT = TL // 128       # 4 local t-tiles; tile tt = seq tt
NCT = C // 128       # 8 c-tiles
LN_EPS = 1e-5

SW = 64.0            # fp8 weight prescale (wq, wk)
SQK = 4.0            # stored scale of q,k  (copy scale SQK/SW = 1/16)
RS = 256.0           # residual stream scale (wo, w2 pre-multiplied by RS)
EXPS = (HD ** -0.5) / (SQK * SQK)   # exp scale: 1/128

# Per-stream AllToAll slot layout (BYTES; buffers are fp8-typed, bf16
# payloads are written through AP.bitcast). Stream g carries seqs {2g,2g+1}.
QOFFH, KOFFH, VOFFH = 0, 32768, 65536
SLOTH = 131072       # qT[128,256]fp8 + kT[128,256]fp8 + v[256,128]bf16
OSLOTH = 65536       # o return slot: [4 units * 64 dims, 128 tokens] bf16

VCW = 512            # lm_head vocab chunk (1KB elem => full DMA bw)
VP = 32256           # vocab padded to 63*512 (host pads Wlm, slices logits)
NVC = VP // VCW      # 63

_prog_cache = {}


def _ap(t, offset, pattern):
    return bass.AP(tensor=t.tensor if isinstance(t, bass.AP) else t, offset=offset, ap=pattern)


def _build(LL=L, debug=False):
    key = (LL, debug)
    if key in _prog_cache:
        return _prog_cache[key]

    nc = bacc.Bacc("TRN2", target_bir_lowering=False, debug=False, num_devices=NCORES)

    x0 = nc.dram_tensor("x0", [TL, C], F32, kind="ExternalInput")
    tri_d = nc.dram_tensor("tri", [128, 128], BF, kind="ExternalInput")
    wq_d = nc.dram_tensor("wq", [L, C, C], FP8, kind="ExternalInput")
    wk_d = nc.dram_tensor("wk", [L, C, C], FP8, kind="ExternalInput")
    wv_d = nc.dram_tensor("wv", [L, C, C], BF, kind="ExternalInput")
    wo_d = nc.dram_tensor("wo", [L, C, C], BF, kind="ExternalInput")
    w1_d = nc.dram_tensor("w1", [L, C, FF], BF, kind="ExternalInput")
    w2_d = nc.dram_tensor("w2", [L, FF, C], BF, kind="ExternalInput")
    bo1_d = nc.dram_tensor("bo1", [L, C], BF, kind="ExternalInput")   # bo*256
    b21_d = nc.dram_tensor("b21", [L, C], BF, kind="ExternalInput")   # b2*256
    # packed per-channel columns (partition-major, see _prep_inputs):
    # [ln1g(L*8) | ln1b | ln2g | ln2b | lnfg(8) | lnfb(8) | b1(L*32)]
    colc_d = nc.dram_tensor("colc", [128, 272], F32, kind="ExternalInput")
    wlm_d = nc.dram_tensor("wlm", [C, VP], BF, kind="ExternalInput")

    logits_d = nc.dram_tensor("logits", [TL, VP], FP16, kind="ExternalOutput")
    dbg_d = None
    if debug:
        dbg_d = nc.dram_tensor("dbg", [LL, TL, C], F32, kind="ExternalOutput")

    with tile.TileContext(nc) as tc:
        import contextlib

        with contextlib.ExitStack() as ctx:
            # SBUF pools (approx per-partition bytes in comments)
            const = ctx.enter_context(tc.tile_pool(name="const", bufs=1))     # ~3K
            xpool = ctx.enter_context(tc.tile_pool(name="x", bufs=1))         # 16K
            xnpool = ctx.enter_context(tc.tile_pool(name="xn", bufs=2))       # 4K
            htpool = ctx.enter_context(tc.tile_pool(name="ht", bufs=1))       # 28K
            wp8 = ctx.enter_context(tc.tile_pool(name="wp8", bufs=4))         # 16K
            wpb = ctx.enter_context(tc.tile_pool(name="wpb", bufs=4))         # 32K
            stg = ctx.enter_context(tc.tile_pool(name="stg", bufs=4))         # 8K
            kqpool = ctx.enter_context(tc.tile_pool(name="kq", bufs=4))       # 8K
            vpool = ctx.enter_context(tc.tile_pool(name="vu", bufs=2))        # 2.5K
            ptpool = ctx.enter_context(tc.tile_pool(name="pt", bufs=4))       # 6K
            otu_pool = ctx.enter_context(tc.tile_pool(name="otu", bufs=2))    # 4K
            otpool = ctx.enter_context(tc.tile_pool(name="ot", bufs=1))       # 8K
            dnpool = ctx.enter_context(tc.tile_pool(name="dn", bufs=2))       # ~1K
            rbpool = ctx.enter_context(tc.tile_pool(name="rb", bufs=2))       # 4K
            gbpool = ctx.enter_context(tc.tile_pool(name="gb", bufs=2))       # 8K
            ugpool = ctx.enter_context(tc.tile_pool(name="ug", bufs=1))       # 32K
            lmw = ctx.enter_context(tc.tile_pool(name="lmw", bufs=2))         # 16K
            lgpool = ctx.enter_context(tc.tile_pool(name="lg", bufs=2))       # 8K
            misc = ctx.enter_context(tc.tile_pool(name="misc", bufs=2))       # ~1K
            psA = ctx.enter_context(tc.tile_pool(name="psA", bufs=5, space="PSUM"))
            ovp = ctx.enter_context(tc.tile_pool(name="ovp", bufs=3, space="PSUM"))
            dram = ctx.enter_context(tc.tile_pool(name="dram", bufs=1, space="DRAM"))

            ident = const.tile([128, 128], BF, name="ident")
            make_identity(nc, ident)
            eps_t = const.tile([128, 1], F32, name="eps")
            nc.vector.memset(eps_t[:], LN_EPS)
            tri = const.tile([128, 128], BF, name="tri")
            nc.gpsimd.dma_start(out=tri[:], in_=tri_d[:])
            ones1 = const.tile([1, 128], BF, name="ones1")
            nc.vector.memset(ones1[:], 1.0)

            a2a_in = [dram.tile([NCORES * SLOTH], FP8, name=f"a2a_in{g}") for g in range(2)]
            a2a_out = [dram.tile([NCORES * SLOTH], FP8, name=f"a2a_out{g}") for g in range(2)]
            o2a_in = [dram.tile([NCORES * OSLOTH], FP8, name=f"o2a_in{g}") for g in range(2)]
            o2a_out = [dram.tile([NCORES * OSLOTH], FP8, name=f"o2a_out{g}") for g in range(2)]

            # persistent residual stream fp32 (carried at 256x): tile tt = seq tt
            x_t = [xpool.tile([128, C], F32, tag=f"x{tt}", name=f"x{tt}") for tt in range(NT)]
            for tt in range(NT):
                nc.sync.dma_start(out=x_t[tt][:], in_=x0[tt * 128:(tt + 1) * 128, :])

            colc = const.tile([128, 272], F32, name="colc")
            nc.gpsimd.dma_start(out=colc[:], in_=colc_d[:])

            def a2a(tin, tout, nbytes):
                nc.gpsimd.collective_compute(
                    "AllToAll",
                    mybir.AluOpType.bypass,
                    replica_groups=[list(range(NCORES))],
                    ins=[_ap(tin, 0, [[1, nbytes]])],
                    outs=[_ap(tout, 0, [[1, nbytes]])],
                )

            def emit_ln(g, goff, boff, tag, fp8_also=False):
                """LN over free dim of stream g's x tiles -> hT [128, NCT, 256]
                bf16 (and optionally an fp8 copy for the Q/K DoubleRow path)."""
                xns = []
                for tl in range(2):
                    xt = x_t[2 * g + tl]
                    stats = misc.tile([128, 2, 6], F32, name="stats", tag="stats")
                    xv = xt[:].rearrange("p (s d) -> p s d", s=2)
                    nc.vector.bn_stats(out=stats[:, 0, :], in_=xv[:, 0, :])
                    nc.vector.bn_stats(out=stats[:, 1, :], in_=xv[:, 1, :])
                    mv = misc.tile([128, 2], F32, name="mv", tag="mv")
                    nc.vector.bn_aggr(out=mv[:], in_=stats[:])
                    rstd = misc.tile([128, 1], F32, name="rstd", tag="rstd")
                    nc.scalar.activation(rstd[:], mv[:, 1:2], AF.Sqrt, bias=eps_t[:])
                    nc.vector.reciprocal(rstd[:], rstd[:])
                    xn = xnpool.tile([128, C], BF, tag="xn", name="xn")
                    nc.vector.tensor_scalar(
                        out=xn[:], in0=xt[:], scalar1=mv[:, 0:1], scalar2=rstd[:],
                        op0=mybir.AluOpType.subtract, op1=mybir.AluOpType.mult,
                    )
                    xns.append(xn)
                hTb = htpool.tile([128, NCT, 256], BF, tag=f"{tag}{g}", name=f"{tag}{g}")
                hT8 = None
                if fp8_also:
                    hT8 = htpool.tile([128, NCT, 256], FP8, tag=f"{tag}8{g}", name=f"{tag}8{g}")
                for ct in range(NCT):
                    pst = psA.tile([128, 256], BF, tag="ps", name="pst")
                    for tl in range(2):
                        nc.tensor.transpose(
                            pst[:, tl * 128:(tl + 1) * 128],
                            xns[tl][:, ct * 128:(ct + 1) * 128],
                            ident[:],
                        )
                    # fused per-channel gain/bias (per-partition after transpose)
                    nc.vector.tensor_scalar(
                        out=hTb[:, ct, :], in0=pst[:],
                        scalar1=colc[:, goff + ct:goff + ct + 1],
                        scalar2=colc[:, boff + ct:boff + ct + 1],
                        op0=mybir.AluOpType.mult, op1=mybir.AluOpType.add,
                    )
                    if fp8_also:
                        nc.gpsimd.tensor_copy(out=hT8[:, ct, :], in_=hTb[:, ct, :])
                return hTb, hT8

            def load_w8(wd, layer_off, col0):
                # [128, 8, 512] fp8: rows 0..1023, cols col0..col0+511
                wt = wp8.tile([128, 8, 512], FP8, tag="w8", name="w8t")
                nc.sync.dma_start(
                    out=wt[:],
                    in_=_ap(wd, layer_off + col0,
                            [[C, 128], [128 * C, 8], [1, 512]]),
                )
                return wt

            def load_wb(wd, layer_off, row0, col0, nrow_t, cols, rows_stride):
                # [128, nrow_t, cols] bf16 tile
                wt = wpb.tile([128, nrow_t, cols], BF, tag="wb", name="wbt")
                nc.sync.dma_start(
                    out=wt[:],
                    in_=_ap(wd, layer_off + row0 * rows_stride + col0,
                            [[rows_stride, 128], [128 * rows_stride, nrow_t], [1, cols]]),
                )
                return wt

            def qkv(g, hTb, hT8, wq2, wk2, wv2):
                """Q/K transposed (fp8 DR) + V natural (bf16) -> a2a_in[g]."""
                for off, wts in ((QOFFH, wq2), (KOFFH, wk2)):
                    for half in range(2):
                        wt = wts[half]
                        pss = [psA.tile([128, 256], F32, tag="ps", name="acc") for _ in range(4)]
                        for j in range(4):
                            for d4 in range(4):
                                nc.tensor.matmul(
                                    pss[d4][:], wt[:, 2 * j:2 * j + 2, d4 * 128:(d4 + 1) * 128],
                                    hT8[:, 2 * j:2 * j + 2, :],
                                    start=(j == 0), stop=(j == 3), perf_mode=DR,
                                )
                        qs = stg.tile([128, 4, 256], FP8, tag="stg", name="qs")
                        for d4 in range(4):
                            nc.scalar.activation(qs[:, d4, :], pss[d4][:], AF.Copy, scale=SQK / SW)
                        nc.gpsimd.dma_start(
                            out=_ap(a2a_in[g], (half * 4) * SLOTH + off,
                                    [[256, 128], [SLOTH, 4], [1, 256]]),
                            in_=qs[:],
                        )
                for nf in range(2):
                    wt = wv2[nf]
                    pss = [psA.tile([128, 512], F32, tag="ps", name="acc") for _ in range(2)]
                    for ct in range(NCT):
                        for tl in range(2):
                            nc.tensor.matmul(
                                pss[tl][:], hTb[:, ct, tl * 128:(tl + 1) * 128],
                                wt[:, ct, :],
                                start=(ct == 0), stop=(ct == NCT - 1),
                            )
                    for tl in range(2):
                        vs = stg.tile([128, 512], BF, tag="stgv", name="vs")
                        nc.vector.tensor_copy(out=vs[:], in_=pss[tl][:])
                        # v region rows are (s_local*128+j), 256B/row (bf16)
                        nc.gpsimd.dma_start(
                            out=_ap(a2a_in[g], (nf * 4) * SLOTH + VOFFH + tl * 128 * 256,
                                    [[256, 128], [SLOTH, 4], [1, 256]]),
                            in_=vs[:].rearrange("p (dt c) -> p dt c", dt=4).bitcast(FP8),
                        )
                a2a(a2a_in[g], a2a_out[g], NCORES * SLOTH)

            def attention(g):
                """4 (seq,parity) units; pairs of units interleave per query
                tile so independent softmax chains alternate on the engines;
                O -> o2a_in[g]."""
                for pair in range(2):
                    kqs, vus, otus = [], [], []
                    for ui in range(2):
                        u = pair * 2 + ui
                        sl, p = u % 2, u // 2
                        kT_u = kqpool.tile([32, 8, 2, 128], FP8, tag="kq", name="kT_u")
                        q_u = kqpool.tile([32, 8, 2, 128], FP8, tag="kq", name="q_u")
                        for two in range(2):
                            nc.sync.dma_start(
                                out=kT_u[:, :, two, :],
                                in_=_ap(a2a_out[g], KOFFH + p * 16384 + two * 8192 + sl * 128,
                                        [[256, 32], [SLOTH, 8], [1, 128]]),
                            )
                            nc.sync.dma_start(
                                out=q_u[:, :, two, :],
                                in_=_ap(a2a_out[g], QOFFH + p * 16384 + two * 8192 + sl * 128,
                                        [[256, 32], [SLOTH, 8], [1, 128]]),
                            )
                        v_u = vpool.tile([128, 8, 80], BF, tag="vu", name="v_u")
                        nc.sync.dma_start(
                            out=v_u[:, :, 0:64].bitcast(FP8),
                            in_=_ap(a2a_out[g], VOFFH + sl * 128 * 256 + p * 128,
                                    [[256, 128], [SLOTH, 8], [1, 128]]),
                        )
                        nc.gpsimd.memset(v_u[:, :, 64:65], 1.0)
                        kqs.append((kT_u, q_u))
                        vus.append(v_u)
                        otus.append(otu_pool.tile([64, 8, 128], BF, tag="otu", name="oTu"))
                    for half in range(2):
                        ovs = [ovp.tile([128, 4, 128], F32, tag="ov", name="ov")
                               for _ in range(2)]
                        for qq in range(4):
                            qt = half * 4 + qq
                            n = qt + 1
                            for ui in range(2):
                                kT_u, q_u = kqs[ui]
                                pT = ptpool.tile([128, 8, 128], BF, tag="pt", name="pT")
                                for base in range(0, n, 4):
                                    cnt = min(4, n - base)
                                    st = psA.tile([128, 4, 128], F32, tag="ps", name="st")
                                    for kk in range(cnt):
                                        nc.tensor.matmul(
                                            st[:, kk, :], kT_u[:, base + kk, :, :],
                                            q_u[:, qt, :, :],
                                            start=True, stop=True, perf_mode=DR,
                                        )
                                    nc.scalar.activation(
                                        pT[:, base:base + cnt, :], st[:, 0:cnt, :],
                                        AF.Exp, scale=EXPS,
                                    )
                                # causal mask: only the diagonal tile is partial
                                nc.vector.tensor_mul(
                                    out=pT[:, qt, :], in0=pT[:, qt, :], in1=tri[:]
                                )
                                for kt in range(n):
                                    nc.tensor.matmul(
                                        ovs[ui][0:65, qq, :], vus[ui][:, kt, 0:65],
                                        pT[:, kt, :],
                                        start=(kt == 0), stop=(kt == n - 1),
                                    )
                        # denominators -> recip -> broadcast (TensorE) -> normalize
                        for ui in range(2):
                            dnr = dnpool.tile([1, 4, 128], BF, tag="dn", name="dnr")
                            with nc.allow_low_precision(reason="softmax denom recip bf16"):
                                nc.vector.reciprocal(dnr[:], ovs[ui][64:65, :, :])
                            rbp = psA.tile([64, 4, 128], F32, tag="ps", name="rbp")
                            nc.tensor.matmul(rbp[:], ones1[0:1, 0:64], dnr[:],
                                             start=True, stop=True)
                            rbs = rbpool.tile([64, 4, 128], F32, tag="rb", name="rbs")
                            nc.scalar.activation(rbs[:], rbp[:], AF.Copy)
                            nc.vector.tensor_mul(
                                out=otus[ui][:, half * 4:half * 4 + 4, :],
                                in0=ovs[ui][0:64, :, :],
                                in1=rbs[:],
                            )
                    for ui in range(2):
                        u = pair * 2 + ui
                        sl, p = u % 2, u // 2
                        nc.gpsimd.dma_start(
                            out=_ap(o2a_in[g], (p * 2 + sl) * 16384,
                                    [[256, 64], [OSLOTH, 8], [1, 256]]),
                            in_=otus[ui][:].bitcast(FP8),
                        )
                a2a(o2a_in[g], o2a_out[g], NCORES * OSLOTH)

            def wo_resid(g, lw, wo2, borow):
                oT = otpool.tile([128, NCT, 256], BF, tag=f"oT{g}", name=f"oT{g}")
                for uu in range(8):
                    for p in range(2):
                        nc.sync.dma_start(
                            out=oT[p * 64:(p + 1) * 64, uu, :].rearrange(
                                "p (s j) -> p s j", s=2).bitcast(FP8),
                            in_=_ap(o2a_out[g], uu * OSLOTH + p * 32768,
                                    [[256, 64], [16384, 2], [1, 256]]),
                        )
                for nf in range(2):
                    wt = wo2[nf]
                    pss = [psA.tile([128, 512], F32, tag="ps", name="acc") for _ in range(2)]
                    for ct in range(NCT):
                        for tl in range(2):
                            nc.tensor.matmul(
                                pss[tl][:], oT[:, ct, tl * 128:(tl + 1) * 128],
                                wt[:, ct, :],
                                start=(ct == 0), stop=False,
                            )
                    for tl in range(2):
                        nc.tensor.matmul(
                            pss[tl][:], ones1[0:1, :], borow[0:1, nf * 512:(nf + 1) * 512],
                            start=False, stop=True,
                        )
                        xs = x_t[2 * g + tl][:, nf * 512:(nf + 1) * 512]
                        nc.vector.tensor_add(out=xs, in0=xs, in1=pss[tl][:])

            def do_ffn(g, lw, h2T):
                """Full FFN for one stream (loads its own weight tiles)."""
                ug = ugpool.tile([128, FF // 128, 256], BF, tag=f"ug{g}", name=f"ug{g}")
                for fg in range(8):
                    w1t = load_wb(w1_d, lw * C * FF, 0, fg * 512, 8, 512, FF)
                    pss = [psA.tile([128, 256], F32, tag="ps", name="acc") for _ in range(4)]
                    for ct in range(NCT):
                        for f4 in range(4):
                            nc.tensor.matmul(
                                pss[f4][:], w1t[:, ct, f4 * 128:(f4 + 1) * 128],
                                h2T[:, ct, :],
                                start=(ct == 0), stop=(ct == NCT - 1),
                            )
                    for f4 in range(4):
                        ft = fg * 4 + f4
                        nc.scalar.activation(
                            ug[:, ft, :], pss[f4][:], AF.Gelu,
                            bias=colc[:, 144 + lw * 32 + ft:144 + lw * 32 + ft + 1],
                        )
                b2row = gbpool.tile([1, C], BF, tag="b2", name="b2row")
                nc.gpsimd.dma_start(out=b2row[:], in_=b21_d[lw:lw + 1, :])
                for nf in range(2):
                    pss2 = [psA.tile([128, 512], F32, tag="ps", name="acc2")
                            for _ in range(2)]
                    for q4 in range(4):
                        w2t = load_wb(w2_d, lw * FF * C, q4 * 1024, nf * 512, 8, 512, C)
                        for jj in range(8):
                            ctf = q4 * 8 + jj
                            for tl in range(2):
                                nc.tensor.matmul(
                                    pss2[tl][:],
                                    ug[:, ctf, tl * 128:(tl + 1) * 128],
                                    w2t[:, jj, :],
                                    start=(ctf == 0), stop=False,
                                )
                    for tl in range(2):
                        nc.tensor.matmul(
                            pss2[tl][:], ones1[0:1, :], b2row[0:1, nf * 512:(nf + 1) * 512],
                            start=False, stop=True,
                        )
                        xs = x_t[2 * g + tl][:, nf * 512:(nf + 1) * 512]
                        nc.vector.tensor_add(out=xs, in0=xs, in1=pss2[tl][:])

            for l in range(LL):
                lw = l % L
                # stream A: LN1 + QKV + A2A; stream B's deferred FFN(l-1)
                # fills stream A's first collective window
                hA, hA8 = emit_ln(0, lw * 8, 32 + lw * 8, "hT", fp8_also=True)
                wq2 = [load_w8(wq_d, lw * C * C, half * 512) for half in range(2)]
                wk2 = [load_w8(wk_d, lw * C * C, half * 512) for half in range(2)]
                wv2 = [load_wb(wv_d, lw * C * C, 0, nf * 512, 8, 512, C) for nf in range(2)]
                qkv(0, hA, hA8, wq2, wk2, wv2)
                hB, hB8 = emit_ln(1, lw * 8, 32 + lw * 8, "hT", fp8_also=True)
                qkv(1, hB, hB8, wq2, wk2, wv2)

                wo2 = [load_wb(wo_d, lw * C * C, 0, nf * 512, 8, 512, C) for nf in range(2)]
                borow = gbpool.tile([1, C], BF, tag="bo", name="borow")
                nc.gpsimd.dma_start(out=borow[:], in_=bo1_d[lw:lw + 1, :])
                attention(0)
                attention(1)

                wo_resid(0, lw, wo2, borow)
                h2A, _ = emit_ln(0, 64 + lw * 8, 96 + lw * 8, "h2T")
                wo_resid(1, lw, wo2, borow)
                h2B, _ = emit_ln(1, 64 + lw * 8, 96 + lw * 8, "h2T")

                do_ffn(0, lw, h2A)
                do_ffn(1, lw, h2B)

                if debug:
                    for tt in range(NT):
                        nc.sync.dma_start(
                            out=dbg_d[l, tt * 128:(tt + 1) * 128, :], in_=x_t[tt][:]
                        )

            # ---- final LN + token-sharded lm_head over full (padded) vocab ----
            hfs = [emit_ln(g, 128, 136, "hfT")[0] for g in range(2)]

            for vc in range(NVC):
                wt = lmw.tile([128, 8, VCW], BF, tag="lmw", name="lmwt")
                nc.sync.dma_start(
                    out=wt[:],
                    in_=_ap(wlm_d, vc * VCW, [[VP, 128], [128 * VP, 8], [1, VCW]]),
                )
                lg = lgpool.tile([128, 4, VCW], FP16, tag="lg", name="lg")
                for ts in range(NT):
                    hf = hfs[ts // 2]
                    tl = ts % 2
                    ps = psA.tile([128, VCW], F32, tag="ps", name="lmacc")
                    for ct in range(NCT):
                        nc.tensor.matmul(
                            ps[:], hf[:, ct, tl * 128:(tl + 1) * 128],
                            wt[:, ct, :],
                            start=(ct == 0), stop=(ct == NCT - 1),
                        )
                    if ts % 2 == 0:
                        nc.scalar.activation(lg[:, ts, :], ps[:], AF.Copy)
                    else:
                        nc.vector.tensor_copy(out=lg[:, ts, :], in_=ps[:])
                nc.sync.dma_start(
                    out=_ap(logits_d, vc * VCW, [[VP, 128], [128 * VP, 4], [1, VCW]]),
                    in_=lg[:],
                )

    nc.compile()
    _prog_cache[key] = nc
    return nc


def _pack_colc(f):
    # [128, 272] f32, partition-major packed per-channel constants
    out = np.zeros((128, 272), dtype=np.float32)

    def cols(a):  # [L, C] -> [128, L*8]: out[p, l*8+ct] = a[l, ct*128+p]
        return a.reshape(L, NCT, 128).transpose(2, 0, 1).reshape(128, L * NCT)

    out[:, 0:32] = cols(f["ln1_g"].astype(np.float32))
    out[:, 32:64] = cols(f["ln1_b"].astype(np.float32))
    out[:, 64:96] = cols(f["ln2_g"].astype(np.float32))
    out[:, 96:128] = cols(f["ln2_b"].astype(np.float32))
    out[:, 128:136] = f["lnf_g"].astype(np.float32).reshape(NCT, 128).T
    out[:, 136:144] = f["lnf_b"].astype(np.float32).reshape(NCT, 128).T
    out[:, 144:272] = f["b1"].astype(np.float32).reshape(L, 32, 128).transpose(2, 0, 1).reshape(128, L * 32)
    return np.ascontiguousarray(out)


def _prep_inputs(inputs):
    f = {k: np.asarray(v) for k, v in inputs.items()}
    idx = f["idx"].astype(np.int64)
    emb = f["emb"].astype(np.float32)
    pos = f["pos_enc"].astype(np.float32)
    # residual stream carried at 256x (LN is affine-invariant; Wo/W2 are
    # pre-scaled by 256 so their PSUM deltas add in without a rescale pass)
    x_full = RS * (emb[idx] + pos[None, :, :])          # [B, T, C] f32

    fp8w = lambda a: np.ascontiguousarray(np.asarray(a, dtype=np.float32) * SW).astype(FP8NP)
    bfw = lambda a, s=1.0: np.ascontiguousarray(np.asarray(a, dtype=np.float32) * s).astype(BF16NP)
    shared = {
        "wq": fp8w(f["Wq"]),
        "wk": fp8w(f["Wk"]),
        "wv": bfw(f["Wv"]),
        "wo": bfw(f["Wo"], RS),
        "w1": bfw(f["W1"]),
        "w2": bfw(f["W2"], RS),
        "bo1": np.ascontiguousarray(f["bo"] * RS).astype(BF16NP),
        "b21": np.ascontiguousarray(f["b2"] * RS).astype(BF16NP),
        "colc": _pack_colc(f),
        "wlm": bfw(np.pad(f["Wlm"].astype(np.float32), ((0, 0), (0, VP - V)))),
        "tri": np.triu(np.ones((128, 128), dtype=np.float32)).astype(BF16NP),
    }

    in_maps = []
    for c in range(NCORES):
        x0_c = np.ascontiguousarray(
            x_full[:, 128 * c:128 * (c + 1), :].reshape(TL, C), dtype=np.float32
        )
        im = dict(shared)
        im["x0"] = x0_c
        in_maps.append(im)
    return in_maps


def kernel(**inputs):
    nc = _build()
    in_maps = _prep_inputs(inputs)
    res = run_bass_kernel_spmd(nc, in_maps, list(range(NCORES)))
    blm = np.asarray(inputs["blm"], dtype=np.float32)
    # core r holds rows (s*128 + j) = token 128r+j of seq s, full vocab
    full = np.empty((B, T, V), dtype=np.float32)
    for r in range(NCORES):
        lr = np.asarray(res.results[r]["logits"], dtype=np.float32).reshape(SEQ, 128, VP)
        full[:, 128 * r:128 * (r + 1), :] = lr[:, :, :V]
    full += blm[None, None, :]
    return full
